# revision 1
# baseline (speedup 1.0000x reference)
"""Trainium2 Bass kernel for the LSTM+dense+softmax model.

Model (see reference): x[T=512, B=256, IN=256] -> LSTM(H=128) last hidden
-> dense(OUT=1000) -> softmax. Data-parallel over batch across 8 cores
(32 batch elements per core), weights replicated.

Layout: recurrent state is kept transposed [H=128 partitions, batch] so the
per-step W_hh matmuls, gate nonlinearities and cell update all run at full
partition width with no transposes. Gate pre-activations for 4 consecutive
steps share one PSUM bank: W_ih*x contributions (+bias) are accumulated
ahead of time, W_hh*h is added when the step arrives, and ScalarE applies
sigmoid/tanh directly out of PSUM.
"""

import numpy as np

import concourse.bacc as bacc
import concourse.mybir as mybir
import concourse.tile as tile
from concourse.bass_utils import run_bass_kernel_spmd

SEQ = 512
B = 256
IN = 256
H = 128
OUT = 1000
N_CORES = 8
BC = B // N_CORES  # 32 batch per core
KT = IN // H  # 2 k-tiles for the input projection
G4 = 4  # gate order in this kernel: i, f, o, g  (torch order i,f,g,o)
PERM = [0, 1, 3, 2]  # torch gate block -> our gate slot
SPB = 4  # steps per PSUM bank group (4*4*32 fp32 = one 2KB bank)
AHEAD = 4  # bank groups of x-projection lookahead
CH = 32  # timesteps per streamed x chunk

F32 = mybir.dt.float32
BF16 = mybir.dt.bfloat16

import os as _os
H_BF16 = _os.environ.get("LSTM_H_BF16", "0") == "1"  # W_hh*h path in bf16
X_BF16 = _os.environ.get("LSTM_X_BF16", "0") == "1"  # W_ih*x (+bias) path in bf16
REP = int(_os.environ.get("LSTM_REP", "1"))  # timing amplification (bench only)

_CACHE = {}


def _build(T):
    ngrp = T // SPB
    ch = min(CH, T)
    HD = BF16 if H_BF16 else F32
    XD = BF16 if X_BF16 else F32
    nc = bacc.Bacc("TRN2", target_bir_lowering=False, debug=False)

    xT = nc.declare_dram_parameter("xT", [H, KT, T, BC], XD, isOutput=False)
    whhT = nc.declare_dram_parameter("whhT", [H, G4, H], HD, isOutput=False)
    wihT = nc.declare_dram_parameter("wihT", [H, KT, G4, H], XD, isOutput=False)
    bias4 = nc.declare_dram_parameter("bias4", [G4, H], XD, isOutput=False)
    ind4 = nc.declare_dram_parameter("ind4", [G4, SPB * G4 * BC], XD, isOutput=False)
    wdT = nc.declare_dram_parameter("wdT", [H, OUT], F32, isOutput=False)
    bd = nc.declare_dram_parameter("bd", [1, OUT], F32, isOutput=False)
    out = nc.declare_dram_parameter("out", [BC, OUT], F32, isOutput=True)

    NSPLIT = 512  # dense tail: first PSUM bank columns
    NREST = OUT - NSPLIT

    with tile.TileContext(nc) as tc:
        with (
            tc.tile_pool(name="const", bufs=1) as constp,
            tc.tile_pool(name="xs", bufs=3) as xpool,
            tc.tile_pool(name="state", bufs=1) as state,
            tc.tile_pool(name="work", bufs=3) as work,
        ):
            whh_s = constp.tile([H, G4, H], HD)
            wih_s = constp.tile([H, KT, G4, H], XD)
            bias_s = constp.tile([G4, H], XD)
            ind_s = constp.tile([G4, SPB * G4 * BC], XD)
            wd_s = constp.tile([H, OUT], F32)
            bd_s = constp.tile([1, OUT], F32)
            ones1 = constp.tile([1, BC], F32)
            nc.gpsimd.dma_start(whh_s[:], whhT[:])
            nc.gpsimd.dma_start(wih_s[:], wihT[:])
            nc.gpsimd.dma_start(bias_s[:], bias4[:])
            nc.gpsimd.dma_start(ind_s[:], ind4[:])
            nc.gpsimd.dma_start(wd_s[:], wdT[:])
            nc.gpsimd.dma_start(bd_s[:], bd[:])
            nc.vector.memset(ones1[:], 1.0)

            # persistent state: h transposed [H, BC].
            # W = [sig(i) sig(f) sig(o) sig(2g) | c]: the sigmoid of all 4
            # (pre-scaled) gates lands in W[:,0:128] right next to the cell
            # state c in W[:,128:160], so [i|f] (x) [sig2g|c] is one
            # contiguous 64-wide multiply. tanh(g) = 2*sig(2g)-1 is folded
            # into the cell update (g weights are pre-doubled on the host).
            hT = state.tile([H, BC], HD)
            hT32 = state.tile([H, BC], F32)
            W = state.tile([H, 5 * BC], F32)

            nchunk = (T + ch - 1) // ch
            xtiles = [None] * nchunk

            def ensure_chunk(ci):
                if xtiles[ci] is None:
                    xt = xpool.tile([H, KT, ch, BC], XD)
                    nc.gpsimd.dma_start(
                        xt[:], xT[:, :, ci * ch : (ci + 1) * ch, :]
                    )
                    xtiles[ci] = xt

            for _rep in range(REP):
              if True:
                xtiles = [None] * nchunk
                nc.vector.memset(hT[:], 0.0)
                nc.vector.memset(W[:], 0.0)
                psump_cm = tc.tile_pool(name=f"psum{_rep}", bufs=AHEAD + 2, space="PSUM")
                psump = psump_cm.__enter__()
                pstiles = [None] * ngrp

                def emit_xproj(g):
                    # accumulate W_ih*x (+ bias) for the 4 steps of group g
                    t0 = g * SPB
                    ci = t0 // ch
                    ensure_chunk(ci)
                    xt = xtiles[ci]
                    s0 = t0 - ci * ch
                    ps = psump.tile([H, SPB, G4, BC], F32)
                    pstiles[g] = ps
                    # bias first: the ONE start=True matmul covering the whole
                    # bank (start=True clears has_written bank-wide, so it must
                    # be the single first writer; everything after accumulates)
                    nc.tensor.matmul(
                        ps[:].rearrange("p a g b -> p (a g b)"),
                        bias_s[:],
                        ind_s[:],
                        start=True,
                        stop=False,
                        skip_group_check=True,
                    )
                    for gi in range(G4):
                        for kt in range(KT):
                            nc.tensor.matmul(
                                ps[:, :, gi, :],
                                wih_s[:, kt, gi, :],
                                xt[:, kt, s0 : s0 + SPB, :],
                                start=False,
                                stop=False,
                                skip_group_check=True,
                            )

                for g in range(min(AHEAD, ngrp)):
                    emit_xproj(g)

                for g in range(ngrp):
                    if g + AHEAD < ngrp:
                        emit_xproj(g + AHEAD)
                    ps = pstiles[g]
                    for s in range(SPB):
                        # W_hh * h into the gate bank (critical path).
                        # g-gate (slot 3) first so tanh(g) can start while the
                        # i/f/o matmuls are still streaming.
                        for gi in (3, 0, 1, 2):
                            nc.tensor.matmul(
                                ps[:, s, gi, :],
                                whh_s[:, gi, :],
                                hT[:],
                                start=False,
                                stop=(gi == 2),
                                skip_group_check=True,
                            )
                        prod = work.tile([H, 2 * BC], F32)
                        tct = work.tile([H, BC], F32)
                        # sigmoid of all 4 gates in one op (g pre-scaled by 2)
                        nc.scalar.activation(
                            W[:, 0 : 4 * BC].rearrange("p (g b) -> p g b", g=4),
                            ps[:, s, :, :],
                            mybir.ActivationFunctionType.Sigmoid,
                        )
                        # prod = [i*sig2g | f*c]
                        nc.vector.tensor_mul(
                            prod[:], W[:, 0 : 2 * BC], W[:, 3 * BC : 5 * BC]
                        )
                        # c = i*(2*sig2g - 1) + f*c = 2*prod0 - i + prod1
                        nc.vector.scalar_tensor_tensor(
                            tct[:], prod[:, 0:BC], 2.0, W[:, 0:BC],
                            op0=mybir.AluOpType.mult,
                            op1=mybir.AluOpType.subtract,
                        )
                        nc.vector.tensor_add(
                            W[:, 4 * BC : 5 * BC], tct[:], prod[:, BC : 2 * BC]
                        )
                        nc.scalar.activation(
                            tct[:],
                            W[:, 4 * BC : 5 * BC],
                            mybir.ActivationFunctionType.Tanh,
                        )
                        t_glob = g * SPB + s
                        if t_glob == T - 1:
                            nc.vector.tensor_mul(
                                hT32[:], W[:, 2 * BC : 3 * BC], tct[:]
                            )
                        else:
                            nc.vector.tensor_mul(
                                hT[:], W[:, 2 * BC : 3 * BC], tct[:]
                            )
                    pstiles[g] = None

                psump_cm.__exit__(None, None, None)
            # dense + softmax tail
            with tc.tile_pool(name="psd", bufs=2, space="PSUM") as psumd:
                lA = psumd.tile([BC, NSPLIT], F32)
                lB = psumd.tile([BC, NREST], F32)
                nc.tensor.matmul(
                    lA[:], hT32[:], wd_s[:, 0:NSPLIT], start=True, stop=False,
                    skip_group_check=True,
                )
                nc.tensor.matmul(
                    lA[:], ones1[:], bd_s[:, 0:NSPLIT], start=False, stop=True,
                    skip_group_check=True,
                )
                nc.tensor.matmul(
                    lB[:], hT32[:], wd_s[:, NSPLIT:OUT], start=True, stop=False,
                    skip_group_check=True,
                )
                nc.tensor.matmul(
                    lB[:], ones1[:], bd_s[:, NSPLIT:OUT], start=False, stop=True,
                    skip_group_check=True,
                )
                mA = work.tile([BC, 1], F32)
                mB = work.tile([BC, 1], F32)
                mneg = work.tile([BC, 1], F32)
                sA = work.tile([BC, 1], F32)
                sB = work.tile([BC, 1], F32)
                stot = work.tile([BC, 1], F32)
                rec = work.tile([BC, 1], F32)
                sm = work.tile([BC, OUT], F32)
                nc.vector.reduce_max(mA[:], lA[:], axis=mybir.AxisListType.X)
                nc.vector.reduce_max(mB[:], lB[:], axis=mybir.AxisListType.X)
                nc.vector.tensor_max(mA[:], mA[:], mB[:])
                nc.vector.tensor_scalar_mul(mneg[:], mA[:], -1.0)
                nc.scalar.activation(
                    sm[:, 0:NSPLIT], lA[:], mybir.ActivationFunctionType.Exp,
                    bias=mneg[:], accum_out=sA[:],
                )
                nc.scalar.activation(
                    sm[:, NSPLIT:OUT], lB[:], mybir.ActivationFunctionType.Exp,
                    bias=mneg[:], accum_out=sB[:],
                )
                nc.vector.tensor_add(stot[:], sA[:], sB[:])
                nc.vector.reciprocal(rec[:], stot[:])
                nc.vector.tensor_scalar_mul(sm[:], sm[:], rec[:])
                nc.gpsimd.dma_start(out[:], sm[:])

    nc.compile()
    return nc


def _get_nc(T):
    key = (T, REP, H_BF16, X_BF16)
    if key not in _CACHE:
        _CACHE[key] = _build(T)
    return _CACHE[key]


def prep_inputs(x, w_ih, w_hh, b_ih, b_hh, w_dense, b_dense):
    import ml_dtypes
    xd = ml_dtypes.bfloat16 if X_BF16 else np.float32
    hd = ml_dtypes.bfloat16 if H_BF16 else np.float32
    T = x.shape[0]
    x = np.ascontiguousarray(x, dtype=np.float32)
    # xT[k, kt, t, b] = x[t, b, kt*128+k]
    xt_all = np.ascontiguousarray(
        x.reshape(T, B, KT, H).transpose(3, 2, 0, 1).astype(xd)
    )
    whhT = np.ascontiguousarray(
        w_hh.reshape(4, H, H)[PERM].transpose(2, 0, 1).astype(hd)
    )
    wihT = np.ascontiguousarray(
        w_ih.reshape(4, H, KT, H)[PERM].transpose(3, 2, 0, 1).astype(xd)
    )
    bias4 = (b_ih + b_hh).reshape(4, H)[PERM].astype(np.float32)
    # pre-scale the g gate (slot 3) by 2: tanh(x) = 2*sigmoid(2x) - 1
    whhT = whhT.copy(); wihT = wihT.copy()
    whhT[:, 3, :] = whhT[:, 3, :] * np.asarray(2.0, whhT.dtype)
    wihT[:, :, 3, :] = wihT[:, :, 3, :] * np.asarray(2.0, wihT.dtype)
    bias4[3] *= 2.0
    bias4 = np.ascontiguousarray(bias4.astype(xd))
    # ind4[g, n] for n = s*(G4*BC) + gq*BC + b  -> 1.0 iff gq == g
    ind4 = np.zeros((G4, SPB * G4 * BC), dtype=xd)
    nidx = np.arange(SPB * G4 * BC)
    gq = (nidx // BC) % G4
    for g in range(G4):
        ind4[g, gq == g] = 1.0
    wdT = np.ascontiguousarray(w_dense.T, dtype=np.float32)
    bd = np.ascontiguousarray(b_dense.reshape(1, OUT), dtype=np.float32)

    in_maps = []
    for c in range(N_CORES):
        in_maps.append(
            {
                "xT": np.ascontiguousarray(xt_all[:, :, :, c * BC : (c + 1) * BC]),
                "whhT": whhT,
                "wihT": wihT,
                "bias4": bias4,
                "ind4": ind4,
                "wdT": wdT,
                "bd": bd,
            }
        )
    return in_maps


def kernel(x, w_ih, w_hh, b_ih, b_hh, w_dense, b_dense):
    x = np.asarray(x)
    T = x.shape[0]
    nc = _get_nc(T)
    in_maps = prep_inputs(
        np.asarray(x), np.asarray(w_ih), np.asarray(w_hh),
        np.asarray(b_ih), np.asarray(b_hh),
        np.asarray(w_dense), np.asarray(b_dense),
    )
    res = run_bass_kernel_spmd(nc, in_maps, list(range(N_CORES)))
    return np.concatenate(
        [res.results[c]["out"] for c in range(N_CORES)], axis=0
    ).astype(np.float32)



# revision 35
# speedup vs baseline: 3196.2440x; 3196.2440x over previous
"""Trainium2 Bass kernel for the LSTM+dense+softmax model.

Model (see reference): x[T=512, B=256, IN=256] -> LSTM(H=128) last hidden
-> dense(OUT=1000) -> softmax. Data-parallel over batch across 8 cores
(32 batch elements per core), weights replicated.

Layout: recurrent state is kept transposed [H=128 partitions, batch] so the
per-step W_hh matmuls, gate nonlinearities and cell update all run at full
partition width with no transposes. Gate pre-activations for 4 consecutive
steps share one PSUM bank: W_ih*x contributions (+bias) are accumulated
ahead of time, W_hh*h is added when the step arrives, and ScalarE applies
sigmoid/tanh directly out of PSUM.

Both matmul paths run in bfloat16 (fp32 matmuls cost 4 PE cycles/row and
are emitted as two half-speed passes; bf16 costs 1): measured 2.0x faster
end to end at rel_err ~1e-3 vs the fp32 reference. The recurrence is
latency-bound (~2.1us/step serial chain: 4x W_hh matmul -> sigmoid ->
3 DVE ops -> tanh -> h-mul); alternative schedules (finer xproj slicing,
wait-on-matmul-instead-of-ldweights, semaphore-throttled lookahead) all
measured slower on hardware.
"""

import numpy as np

import concourse.bacc as bacc
import concourse.mybir as mybir
import concourse.tile as tile
from concourse.bass_utils import run_bass_kernel_spmd

SEQ = 512
B = 256
IN = 256
H = 128
OUT = 1000
N_CORES = 8
BC = B // N_CORES  # 32 batch per core
KT = IN // H  # 2 k-tiles for the input projection
G4 = 4  # gate order in this kernel: i, f, o, g  (torch order i,f,g,o)
PERM = [0, 1, 3, 2]  # torch gate block -> our gate slot
SPB = 4  # steps per PSUM bank group (4*4*32 fp32 = one 2KB bank)
AHEAD = 4  # bank groups of x-projection lookahead
CH = 32  # timesteps per streamed x chunk

F32 = mybir.dt.float32
BF16 = mybir.dt.bfloat16

import os as _os
H_BF16 = _os.environ.get("LSTM_H_BF16", "1") == "1"  # W_hh*h path in bf16
X_BF16 = _os.environ.get("LSTM_X_BF16", "1") == "1"  # W_ih*x (+bias) path in bf16
REP = int(_os.environ.get("LSTM_REP", "1"))  # timing amplification (bench only)

_CACHE = {}


def _build(T):
    ngrp = T // SPB
    ch = min(CH, T)
    HD = BF16 if H_BF16 else F32
    XD = BF16 if X_BF16 else F32
    nc = bacc.Bacc("TRN2", target_bir_lowering=False, debug=False)

    xT = nc.declare_dram_parameter("xT", [H, KT, T, BC], XD, isOutput=False)
    whhT = nc.declare_dram_parameter("whhT", [H, G4, H], HD, isOutput=False)
    wihT = nc.declare_dram_parameter("wihT", [H, KT, G4, H], XD, isOutput=False)
    bias4 = nc.declare_dram_parameter("bias4", [G4, H], XD, isOutput=False)
    ind4 = nc.declare_dram_parameter("ind4", [G4, SPB * G4 * BC], XD, isOutput=False)
    wdT = nc.declare_dram_parameter("wdT", [H, OUT], F32, isOutput=False)
    bd = nc.declare_dram_parameter("bd", [1, OUT], F32, isOutput=False)
    out = nc.declare_dram_parameter("out", [BC, OUT], F32, isOutput=True)

    NSPLIT = 512  # dense tail: first PSUM bank columns
    NREST = OUT - NSPLIT

    with tile.TileContext(nc) as tc:
        with (
            tc.tile_pool(name="const", bufs=1) as constp,
            tc.tile_pool(name="xs", bufs=3) as xpool,
            tc.tile_pool(name="state", bufs=1) as state,
            tc.tile_pool(name="work", bufs=3) as work,
        ):
            whh_s = constp.tile([H, G4, H], HD)
            wih_s = constp.tile([H, KT, G4, H], XD)
            bias_s = constp.tile([G4, H], XD)
            ind_s = constp.tile([G4, SPB * G4 * BC], XD)
            wd_s = constp.tile([H, OUT], F32)
            bd_s = constp.tile([1, OUT], F32)
            ones1 = constp.tile([1, BC], F32)
            nc.gpsimd.dma_start(whh_s[:], whhT[:])
            nc.gpsimd.dma_start(wih_s[:], wihT[:])
            nc.gpsimd.dma_start(bias_s[:], bias4[:])
            nc.gpsimd.dma_start(ind_s[:], ind4[:])
            nc.gpsimd.dma_start(wd_s[:], wdT[:])
            nc.gpsimd.dma_start(bd_s[:], bd[:])
            nc.vector.memset(ones1[:], 1.0)

            # persistent state: h transposed [H, BC].
            # W = [sig(i) sig(f) sig(o) sig(2g) | c]: the sigmoid of all 4
            # (pre-scaled) gates lands in W[:,0:128] right next to the cell
            # state c in W[:,128:160], so [i|f] (x) [sig2g|c] is one
            # contiguous 64-wide multiply. tanh(g) = 2*sig(2g)-1 is folded
            # into the cell update (g weights are pre-doubled on the host).
            hT = state.tile([H, BC], HD)
            hT32 = state.tile([H, BC], F32)
            W = state.tile([H, 5 * BC], F32)

            nchunk = (T + ch - 1) // ch
            xtiles = [None] * nchunk

            def ensure_chunk(ci):
                if xtiles[ci] is None:
                    xt = xpool.tile([H, KT, ch, BC], XD)
                    nc.gpsimd.dma_start(
                        xt[:], xT[:, :, ci * ch : (ci + 1) * ch, :]
                    )
                    xtiles[ci] = xt

            for _rep in range(REP):
              if True:
                xtiles = [None] * nchunk
                nc.vector.memset(hT[:], 0.0)
                nc.vector.memset(W[:], 0.0)
                psump_cm = tc.tile_pool(name=f"psum{_rep}", bufs=AHEAD + 2, space="PSUM")
                psump = psump_cm.__enter__()
                pstiles = [None] * ngrp

                def emit_xproj(g):
                    # accumulate W_ih*x (+ bias) for the 4 steps of group g
                    t0 = g * SPB
                    ci = t0 // ch
                    ensure_chunk(ci)
                    xt = xtiles[ci]
                    s0 = t0 - ci * ch
                    ps = psump.tile([H, SPB, G4, BC], F32)
                    pstiles[g] = ps
                    # bias first: the ONE start=True matmul covering the whole
                    # bank (start=True clears has_written bank-wide, so it must
                    # be the single first writer; everything after accumulates)
                    nc.tensor.matmul(
                        ps[:].rearrange("p a g b -> p (a g b)"),
                        bias_s[:],
                        ind_s[:],
                        start=True,
                        stop=False,
                        skip_group_check=True,
                    )
                    for gi in range(G4):
                        for kt in range(KT):
                            nc.tensor.matmul(
                                ps[:, :, gi, :],
                                wih_s[:, kt, gi, :],
                                xt[:, kt, s0 : s0 + SPB, :],
                                start=False,
                                stop=False,
                                skip_group_check=True,
                            )

                for g in range(min(AHEAD, ngrp)):
                    emit_xproj(g)

                for g in range(ngrp):
                    if g + AHEAD < ngrp:
                        emit_xproj(g + AHEAD)
                    ps = pstiles[g]
                    for s in range(SPB):
                        # W_hh * h into the gate bank (critical path).
                        # g-gate (slot 3) first so tanh(g) can start while the
                        # i/f/o matmuls are still streaming.
                        for gi in (3, 0, 1, 2):
                            nc.tensor.matmul(
                                ps[:, s, gi, :],
                                whh_s[:, gi, :],
                                hT[:],
                                start=False,
                                stop=(gi == 2),
                                skip_group_check=True,
                            )
                        prod = work.tile([H, 2 * BC], F32)
                        tct = work.tile([H, BC], F32)
                        # sigmoid of all 4 gates in one op (g pre-scaled by 2)
                        nc.scalar.activation(
                            W[:, 0 : 4 * BC].rearrange("p (g b) -> p g b", g=4),
                            ps[:, s, :, :],
                            mybir.ActivationFunctionType.Sigmoid,
                        )
                        # prod = [i*sig2g | f*c]
                        nc.vector.tensor_mul(
                            prod[:], W[:, 0 : 2 * BC], W[:, 3 * BC : 5 * BC]
                        )
                        # c = i*(2*sig2g - 1) + f*c = 2*prod0 - i + prod1
                        nc.vector.scalar_tensor_tensor(
                            tct[:], prod[:, 0:BC], 2.0, W[:, 0:BC],
                            op0=mybir.AluOpType.mult,
                            op1=mybir.AluOpType.subtract,
                        )
                        nc.vector.tensor_add(
                            W[:, 4 * BC : 5 * BC], tct[:], prod[:, BC : 2 * BC]
                        )
                        nc.scalar.activation(
                            tct[:],
                            W[:, 4 * BC : 5 * BC],
                            mybir.ActivationFunctionType.Tanh,
                        )
                        t_glob = g * SPB + s
                        if t_glob == T - 1:
                            nc.vector.tensor_mul(
                                hT32[:], W[:, 2 * BC : 3 * BC], tct[:]
                            )
                        else:
                            nc.vector.tensor_mul(
                                hT[:], W[:, 2 * BC : 3 * BC], tct[:]
                            )
                    pstiles[g] = None

                psump_cm.__exit__(None, None, None)
            # dense + softmax tail
            with tc.tile_pool(name="psd", bufs=2, space="PSUM") as psumd:
                lA = psumd.tile([BC, NSPLIT], F32)
                lB = psumd.tile([BC, NREST], F32)
                nc.tensor.matmul(
                    lA[:], hT32[:], wd_s[:, 0:NSPLIT], start=True, stop=False,
                    skip_group_check=True,
                )
                nc.tensor.matmul(
                    lA[:], ones1[:], bd_s[:, 0:NSPLIT], start=False, stop=True,
                    skip_group_check=True,
                )
                nc.tensor.matmul(
                    lB[:], hT32[:], wd_s[:, NSPLIT:OUT], start=True, stop=False,
                    skip_group_check=True,
                )
                nc.tensor.matmul(
                    lB[:], ones1[:], bd_s[:, NSPLIT:OUT], start=False, stop=True,
                    skip_group_check=True,
                )
                mA = work.tile([BC, 1], F32)
                mB = work.tile([BC, 1], F32)
                mneg = work.tile([BC, 1], F32)
                sA = work.tile([BC, 1], F32)
                sB = work.tile([BC, 1], F32)
                stot = work.tile([BC, 1], F32)
                rec = work.tile([BC, 1], F32)
                sm = work.tile([BC, OUT], F32)
                nc.vector.reduce_max(mA[:], lA[:], axis=mybir.AxisListType.X)
                nc.vector.reduce_max(mB[:], lB[:], axis=mybir.AxisListType.X)
                nc.vector.tensor_max(mA[:], mA[:], mB[:])
                nc.vector.tensor_scalar_mul(mneg[:], mA[:], -1.0)
                nc.scalar.activation(
                    sm[:, 0:NSPLIT], lA[:], mybir.ActivationFunctionType.Exp,
                    bias=mneg[:], accum_out=sA[:],
                )
                nc.scalar.activation(
                    sm[:, NSPLIT:OUT], lB[:], mybir.ActivationFunctionType.Exp,
                    bias=mneg[:], accum_out=sB[:],
                )
                nc.vector.tensor_add(stot[:], sA[:], sB[:])
                nc.vector.reciprocal(rec[:], stot[:])
                nc.vector.tensor_scalar_mul(sm[:], sm[:], rec[:])
                nc.gpsimd.dma_start(out[:], sm[:])

    nc.compile()
    return nc


def _get_nc(T):
    key = (T, REP, H_BF16, X_BF16)
    if key not in _CACHE:
        _CACHE[key] = _build(T)
    return _CACHE[key]


def prep_inputs(x, w_ih, w_hh, b_ih, b_hh, w_dense, b_dense):
    import ml_dtypes
    xd = ml_dtypes.bfloat16 if X_BF16 else np.float32
    hd = ml_dtypes.bfloat16 if H_BF16 else np.float32
    T = x.shape[0]
    x = np.ascontiguousarray(x, dtype=np.float32)
    # xT[k, kt, t, b] = x[t, b, kt*128+k]
    xt_all = np.ascontiguousarray(
        x.reshape(T, B, KT, H).transpose(3, 2, 0, 1).astype(xd)
    )
    whhT = np.ascontiguousarray(
        w_hh.reshape(4, H, H)[PERM].transpose(2, 0, 1).astype(hd)
    )
    wihT = np.ascontiguousarray(
        w_ih.reshape(4, H, KT, H)[PERM].transpose(3, 2, 0, 1).astype(xd)
    )
    bias4 = (b_ih + b_hh).reshape(4, H)[PERM].astype(np.float32)
    # pre-scale the g gate (slot 3) by 2: tanh(x) = 2*sigmoid(2x) - 1
    whhT = whhT.copy(); wihT = wihT.copy()
    whhT[:, 3, :] = whhT[:, 3, :] * np.asarray(2.0, whhT.dtype)
    wihT[:, :, 3, :] = wihT[:, :, 3, :] * np.asarray(2.0, wihT.dtype)
    bias4[3] *= 2.0
    bias4 = np.ascontiguousarray(bias4.astype(xd))
    # ind4[g, n] for n = s*(G4*BC) + gq*BC + b  -> 1.0 iff gq == g
    ind4 = np.zeros((G4, SPB * G4 * BC), dtype=xd)
    nidx = np.arange(SPB * G4 * BC)
    gq = (nidx // BC) % G4
    for g in range(G4):
        ind4[g, gq == g] = 1.0
    wdT = np.ascontiguousarray(w_dense.T, dtype=np.float32)
    bd = np.ascontiguousarray(b_dense.reshape(1, OUT), dtype=np.float32)

    in_maps = []
    for c in range(N_CORES):
        in_maps.append(
            {
                "xT": np.ascontiguousarray(xt_all[:, :, :, c * BC : (c + 1) * BC]),
                "whhT": whhT,
                "wihT": wihT,
                "bias4": bias4,
                "ind4": ind4,
                "wdT": wdT,
                "bd": bd,
            }
        )
    return in_maps


def kernel(x, w_ih, w_hh, b_ih, b_hh, w_dense, b_dense):
    x = np.asarray(x)
    T = x.shape[0]
    nc = _get_nc(T)
    in_maps = prep_inputs(
        np.asarray(x), np.asarray(w_ih), np.asarray(w_hh),
        np.asarray(b_ih), np.asarray(b_hh),
        np.asarray(w_dense), np.asarray(b_dense),
    )
    res = run_bass_kernel_spmd(nc, in_maps, list(range(N_CORES)))
    return np.concatenate(
        [res.results[c]["out"] for c in range(N_CORES)], axis=0
    ).astype(np.float32)


# revision 45
# speedup vs baseline: 3201.0082x; 1.0015x over previous
"""Trainium2 Bass kernel for the LSTM+dense+softmax model.

Model (see reference): x[T=512, B=256, IN=256] -> LSTM(H=128) last hidden
-> dense(OUT=1000) -> softmax. Data-parallel over batch across 8 cores
(32 batch elements per core), weights replicated.

Layout: recurrent state is kept transposed [H=128 partitions, batch] so the
per-step W_hh matmuls, gate nonlinearities and cell update all run at full
partition width with no transposes. Gate pre-activations for 4 consecutive
steps share one PSUM bank: W_ih*x contributions (+bias) are accumulated
ahead of time, W_hh*h is added when the step arrives, and ScalarE applies
sigmoid/tanh directly out of PSUM.

Both matmul paths run in bfloat16 (fp32 matmuls cost 4 PE cycles/row and
are emitted as two half-speed passes; bf16 costs 1): measured 2.0x faster
end to end at rel_err ~1e-3 vs the fp32 reference. The recurrence is
latency-bound (~2.1us/step serial chain: 4x W_hh matmul -> sigmoid ->
3 DVE ops -> tanh -> h-mul); alternative schedules (finer xproj slicing,
wait-on-matmul-instead-of-ldweights, semaphore-throttled lookahead) all
measured slower on hardware.
"""

import numpy as np

import concourse.bacc as bacc
import concourse.mybir as mybir
import concourse.tile as tile
from concourse.bass_utils import run_bass_kernel_spmd

SEQ = 512
B = 256
IN = 256
H = 128
OUT = 1000
N_CORES = 8
BC = B // N_CORES  # 32 batch per core
KT = IN // H  # 2 k-tiles for the input projection
G4 = 4  # gate order in this kernel: i, f, o, g  (torch order i,f,g,o)
PERM = [0, 1, 3, 2]  # torch gate block -> our gate slot
SPB = 4  # steps per PSUM bank group (4*4*32 fp32 = one 2KB bank)
AHEAD = 4  # bank groups of x-projection lookahead
CH = 32  # timesteps per streamed x chunk

F32 = mybir.dt.float32
BF16 = mybir.dt.bfloat16

import os as _os
H_BF16 = _os.environ.get("LSTM_H_BF16", "1") == "1"  # W_hh*h path in bf16
X_BF16 = _os.environ.get("LSTM_X_BF16", "1") == "1"  # W_ih*x (+bias) path in bf16
REP = int(_os.environ.get("LSTM_REP", "1"))  # timing amplification (bench only)

_CACHE = {}


def _build(T):
    ngrp = T // SPB
    ch = min(CH, T)
    HD = BF16 if H_BF16 else F32
    XD = BF16 if X_BF16 else F32
    nc = bacc.Bacc("TRN2", target_bir_lowering=False, debug=False)

    xT = nc.declare_dram_parameter("xT", [H, KT, T, BC], XD, isOutput=False)
    whhT = nc.declare_dram_parameter("whhT", [H, G4, H], HD, isOutput=False)
    wihT = nc.declare_dram_parameter("wihT", [H, KT, G4, H], XD, isOutput=False)
    bias4 = nc.declare_dram_parameter("bias4", [G4, H], XD, isOutput=False)
    ind4 = nc.declare_dram_parameter("ind4", [G4, SPB * G4 * BC], XD, isOutput=False)
    wdT = nc.declare_dram_parameter("wdT", [H, OUT], F32, isOutput=False)
    bd = nc.declare_dram_parameter("bd", [1, OUT], F32, isOutput=False)
    out = nc.declare_dram_parameter("out", [BC, OUT], F32, isOutput=True)

    NSPLIT = 512  # dense tail: first PSUM bank columns
    NREST = OUT - NSPLIT

    with tile.TileContext(nc) as tc:
        with (
            tc.tile_pool(name="const", bufs=1) as constp,
            tc.tile_pool(name="xs", bufs=3) as xpool,
            tc.tile_pool(name="state", bufs=1) as state,
            tc.tile_pool(name="work", bufs=3) as work,
        ):
            whh_s = constp.tile([H, G4, H], HD)
            wih_s = constp.tile([H, KT, G4, H], XD)
            bias_s = constp.tile([G4, H], XD)
            ind_s = constp.tile([G4, SPB * G4 * BC], XD)
            wd_s = constp.tile([H, OUT], F32)
            bd_s = constp.tile([1, OUT], F32)
            ones1 = constp.tile([1, BC], F32)
            nc.gpsimd.dma_start(whh_s[:], whhT[:])
            nc.gpsimd.dma_start(wih_s[:], wihT[:])
            nc.gpsimd.dma_start(bias_s[:], bias4[:])
            nc.gpsimd.dma_start(ind_s[:], ind4[:])
            nc.gpsimd.dma_start(wd_s[:], wdT[:])
            nc.gpsimd.dma_start(bd_s[:], bd[:])
            nc.vector.memset(ones1[:], 1.0)

            # persistent state: h transposed [H, BC].
            # W = [sig(i) sig(f) sig(o) sig(2g) | c]: the sigmoid of all 4
            # (pre-scaled) gates lands in W[:,0:128] right next to the cell
            # state c in W[:,128:160], so [i|f] (x) [sig2g|c] is one
            # contiguous 64-wide multiply. tanh(g) = 2*sig(2g)-1 is folded
            # into the cell update (g weights are pre-doubled on the host).
            # (A 2-op cell update via a duplicated-sigmoid scatter measured
            # dead even on HW: the saved DVE op's fixed cost reappears in
            # the 2x-wider activation. This layout is the local optimum.)
            hT = state.tile([H, BC], HD)
            hT32 = state.tile([H, BC], F32)
            W = state.tile([H, 5 * BC], F32)

            nchunk = (T + ch - 1) // ch
            xtiles = [None] * nchunk

            def ensure_chunk(ci):
                if xtiles[ci] is None:
                    xt = xpool.tile([H, KT, ch, BC], XD)
                    nc.gpsimd.dma_start(
                        xt[:], xT[:, :, ci * ch : (ci + 1) * ch, :]
                    )
                    xtiles[ci] = xt

            for _rep in range(REP):
              if True:
                xtiles = [None] * nchunk
                nc.vector.memset(hT[:], 0.0)
                nc.vector.memset(W[:], 0.0)
                psump_cm = tc.tile_pool(name=f"psum{_rep}", bufs=AHEAD + 2, space="PSUM")
                psump = psump_cm.__enter__()
                pstiles = [None] * ngrp

                def emit_xproj(g):
                    # accumulate W_ih*x (+ bias) for the 4 steps of group g
                    t0 = g * SPB
                    ci = t0 // ch
                    ensure_chunk(ci)
                    xt = xtiles[ci]
                    s0 = t0 - ci * ch
                    ps = psump.tile([H, SPB, G4, BC], F32)
                    pstiles[g] = ps
                    # bias first: the ONE start=True matmul covering the whole
                    # bank (start=True clears has_written bank-wide, so it must
                    # be the single first writer; everything after accumulates)
                    nc.tensor.matmul(
                        ps[:].rearrange("p a g b -> p (a g b)"),
                        bias_s[:],
                        ind_s[:],
                        start=True,
                        stop=False,
                        skip_group_check=True,
                    )
                    for gi in range(G4):
                        for kt in range(KT):
                            nc.tensor.matmul(
                                ps[:, :, gi, :],
                                wih_s[:, kt, gi, :],
                                xt[:, kt, s0 : s0 + SPB, :],
                                start=False,
                                stop=False,
                                skip_group_check=True,
                            )

                for g in range(min(AHEAD, ngrp)):
                    emit_xproj(g)

                for g in range(ngrp):
                    if g + AHEAD < ngrp:
                        emit_xproj(g + AHEAD)
                    ps = pstiles[g]
                    for s in range(SPB):
                        # W_hh * h into the gate bank (critical path).
                        # g-gate (slot 3) first so tanh(g) can start while the
                        # i/f/o matmuls are still streaming.
                        for gi in (3, 0, 1, 2):
                            nc.tensor.matmul(
                                ps[:, s, gi, :],
                                whh_s[:, gi, :],
                                hT[:],
                                start=False,
                                stop=(gi == 2),
                                skip_group_check=True,
                            )
                        prod = work.tile([H, 2 * BC], F32)
                        tct = work.tile([H, BC], F32)
                        # sigmoid of all 4 gates in one op (g pre-scaled by 2)
                        nc.scalar.activation(
                            W[:, 0 : 4 * BC].rearrange("p (g b) -> p g b", g=4),
                            ps[:, s, :, :],
                            mybir.ActivationFunctionType.Sigmoid,
                        )
                        # prod = [i*sig2g | f*c]
                        nc.vector.tensor_mul(
                            prod[:], W[:, 0 : 2 * BC], W[:, 3 * BC : 5 * BC]
                        )
                        # c = i*(2*sig2g - 1) + f*c = 2*prod0 - i + prod1
                        nc.vector.scalar_tensor_tensor(
                            tct[:], prod[:, 0:BC], 2.0, W[:, 0:BC],
                            op0=mybir.AluOpType.mult,
                            op1=mybir.AluOpType.subtract,
                        )
                        nc.vector.tensor_add(
                            W[:, 4 * BC : 5 * BC], tct[:], prod[:, BC : 2 * BC]
                        )
                        nc.scalar.activation(
                            tct[:],
                            W[:, 4 * BC : 5 * BC],
                            mybir.ActivationFunctionType.Tanh,
                        )
                        t_glob = g * SPB + s
                        if t_glob == T - 1:
                            nc.vector.tensor_mul(
                                hT32[:], W[:, 2 * BC : 3 * BC], tct[:]
                            )
                        else:
                            nc.vector.tensor_mul(
                                hT[:], W[:, 2 * BC : 3 * BC], tct[:]
                            )
                    pstiles[g] = None

                psump_cm.__exit__(None, None, None)
            # dense + softmax tail
            with tc.tile_pool(name="psd", bufs=2, space="PSUM") as psumd:
                lA = psumd.tile([BC, NSPLIT], F32)
                lB = psumd.tile([BC, NREST], F32)
                nc.tensor.matmul(
                    lA[:], hT32[:], wd_s[:, 0:NSPLIT], start=True, stop=False,
                    skip_group_check=True,
                )
                nc.tensor.matmul(
                    lA[:], ones1[:], bd_s[:, 0:NSPLIT], start=False, stop=True,
                    skip_group_check=True,
                )
                nc.tensor.matmul(
                    lB[:], hT32[:], wd_s[:, NSPLIT:OUT], start=True, stop=False,
                    skip_group_check=True,
                )
                nc.tensor.matmul(
                    lB[:], ones1[:], bd_s[:, NSPLIT:OUT], start=False, stop=True,
                    skip_group_check=True,
                )
                mA = work.tile([BC, 1], F32)
                mB = work.tile([BC, 1], F32)
                mneg = work.tile([BC, 1], F32)
                sA = work.tile([BC, 1], F32)
                sB = work.tile([BC, 1], F32)
                stot = work.tile([BC, 1], F32)
                rec = work.tile([BC, 1], F32)
                sm = work.tile([BC, OUT], F32)
                nc.vector.reduce_max(mA[:], lA[:], axis=mybir.AxisListType.X)
                nc.vector.reduce_max(mB[:], lB[:], axis=mybir.AxisListType.X)
                nc.vector.tensor_max(mA[:], mA[:], mB[:])
                nc.vector.tensor_scalar_mul(mneg[:], mA[:], -1.0)
                nc.scalar.activation(
                    sm[:, 0:NSPLIT], lA[:], mybir.ActivationFunctionType.Exp,
                    bias=mneg[:], accum_out=sA[:],
                )
                nc.scalar.activation(
                    sm[:, NSPLIT:OUT], lB[:], mybir.ActivationFunctionType.Exp,
                    bias=mneg[:], accum_out=sB[:],
                )
                nc.vector.tensor_add(stot[:], sA[:], sB[:])
                nc.vector.reciprocal(rec[:], stot[:])
                nc.vector.tensor_scalar_mul(sm[:], sm[:], rec[:])
                nc.gpsimd.dma_start(out[:], sm[:])

    nc.compile()
    return nc


def _get_nc(T):
    key = (T, REP, H_BF16, X_BF16)
    if key not in _CACHE:
        _CACHE[key] = _build(T)
    return _CACHE[key]


def prep_inputs(x, w_ih, w_hh, b_ih, b_hh, w_dense, b_dense):
    import ml_dtypes
    xd = ml_dtypes.bfloat16 if X_BF16 else np.float32
    hd = ml_dtypes.bfloat16 if H_BF16 else np.float32
    T = x.shape[0]
    x = np.ascontiguousarray(x, dtype=np.float32)
    # xT[k, kt, t, b] = x[t, b, kt*128+k]
    xt_all = np.ascontiguousarray(
        x.reshape(T, B, KT, H).transpose(3, 2, 0, 1).astype(xd)
    )
    whhT = np.ascontiguousarray(
        w_hh.reshape(4, H, H)[PERM].transpose(2, 0, 1).astype(hd)
    )
    wihT = np.ascontiguousarray(
        w_ih.reshape(4, H, KT, H)[PERM].transpose(3, 2, 0, 1).astype(xd)
    )
    bias4 = (b_ih + b_hh).reshape(4, H)[PERM].astype(np.float32)
    # pre-scale the g gate (slot 3) by 2: tanh(x) = 2*sigmoid(2x) - 1
    whhT = whhT.copy(); wihT = wihT.copy()
    whhT[:, 3, :] = whhT[:, 3, :] * np.asarray(2.0, whhT.dtype)
    wihT[:, :, 3, :] = wihT[:, :, 3, :] * np.asarray(2.0, wihT.dtype)
    bias4[3] *= 2.0
    bias4 = np.ascontiguousarray(bias4.astype(xd))
    # ind4[g, n] for n = s*(G4*BC) + gq*BC + b  -> 1.0 iff gq == g
    ind4 = np.zeros((G4, SPB * G4 * BC), dtype=xd)
    nidx = np.arange(SPB * G4 * BC)
    gq = (nidx // BC) % G4
    for g in range(G4):
        ind4[g, gq == g] = 1.0
    wdT = np.ascontiguousarray(w_dense.T, dtype=np.float32)
    bd = np.ascontiguousarray(b_dense.reshape(1, OUT), dtype=np.float32)

    in_maps = []
    for c in range(N_CORES):
        in_maps.append(
            {
                "xT": np.ascontiguousarray(xt_all[:, :, :, c * BC : (c + 1) * BC]),
                "whhT": whhT,
                "wihT": wihT,
                "bias4": bias4,
                "ind4": ind4,
                "wdT": wdT,
                "bd": bd,
            }
        )
    return in_maps


def kernel(x, w_ih, w_hh, b_ih, b_hh, w_dense, b_dense):
    x = np.asarray(x)
    T = x.shape[0]
    nc = _get_nc(T)
    in_maps = prep_inputs(
        np.asarray(x), np.asarray(w_ih), np.asarray(w_hh),
        np.asarray(b_ih), np.asarray(b_hh),
        np.asarray(w_dense), np.asarray(b_dense),
    )
    res = run_bass_kernel_spmd(nc, in_maps, list(range(N_CORES)))
    return np.concatenate(
        [res.results[c]["out"] for c in range(N_CORES)], axis=0
    ).astype(np.float32)
